# revision 25
# baseline (speedup 1.0000x reference)
"""BiLSTM Trainium2 kernel, v6 — device-side gather from a sharded table.

Sharding: 8 cores = 4 batch quarters x 2 directions (as v4: 32 batch rows
per core, one direction each, 3-layer 96-row wavefront, For_i tick loop,
host-side dense head).

v6 versus v4: instead of shipping each core its pre-gathered X^T
(3.2MB fp8), ship the fp8 embedding table SHARDED over the 8 cores
(1.25MB/core) plus per-core token ids (64KB) and cap-feature rows (64KB),
and rebuild X^T on device: AllGather the table into DRAM, indirect-DMA
gather the 16000 token rows, PE-transpose to feature-major bf16, write to
a DRAM X^T buffer that the (unchanged) For_i tick loop streams from.
The weights also ship sharded by direction (0.37MB/core, AllGather over
each direction's 4 cores). Per-core upload drops ~6.1MB -> ~1.75MB; the
warm wall is upload-bound at ~55MB/s.

Numerics as v4: fp8 e4m3 uploads pre-scaled by 8, dequantized to bf16 on
device (X^T stays x8; W0's x-facing chunks carry 1/64, other chunks 1/8).
"""

import sys

import numpy as np

sys.path.insert(0, "/opt/trn_rl_repo")

from contextlib import ExitStack

import concourse.bacc as bacc
import concourse.mybir as mybir
import concourse.tile as tile
from concourse.bass import IndirectOffsetOnAxis, ds, ts
from concourse.bass_utils import run_bass_kernel_spmd
from concourse.masks import make_identity

FP32 = mybir.dt.float32
BF16 = mybir.dt.bfloat16
FP8 = mybir.dt.float8e4
INT32 = mybir.dt.int32

VOCAB, EMB, T_FULL, B_FULL, H, NC_OUT = 50000, 200, 500, 128, 256, 6
BQ = 32          # batch per core
G4 = 4 * H       # 1024 gate width
HALF = 512       # matmul N per PSUM bank
XTA = 128        # X^T rows in first chunk
XTB = 76         # X^T rows in second chunk (72 emb + 3 cap + 1 ones)
ESH = VOCAB // 8  # embedding-table rows per core
WSH = 372        # weight-shard rows per core (4 * 372 = 1488 per direction)

# gate slices after host permutation [f, i, o, j]
SL_F = slice(0, 256)
SL_I = slice(256, 512)
SL_O = slice(512, 768)
SL_J = slice(768, 1024)


def _build_program(T, esh, delta, has_bias=True):
    """esh = embedding-shard rows per core (compacted vocab / 8);
    delta = int4 embedding quantizer step (values = (code-7.5)*delta)."""
    TOK = BQ * T
    NTILE = TOK // 128
    assert T >= 8 and T % 2 == 0 and TOK % 128 == 0

    nc = bacc.Bacc(None, target_bir_lowering=False, debug=False)

    embsh = nc.dram_tensor("embsh", [esh, EMB // 2], mybir.dt.uint8,
                           kind="ExternalInput")
    widx = nc.dram_tensor("widx", [128, NTILE], mybir.dt.uint16,
                          kind="ExternalInput")
    caph = nc.dram_tensor("caph", [4, TOK], FP8, kind="ExternalInput")
    wcat = nc.dram_tensor("wcat", [WSH, G4], FP8, kind="ExternalInput")
    out = nc.dram_tensor("out", [128, 2 * BQ], BF16, kind="ExternalOutput")

    wr = 513 if has_bias else 512
    assert 460 + 2 * wr <= 4 * WSH

    with tile.TileContext(nc) as tc, ExitStack() as ctx:
        dram = ctx.enter_context(tc.tile_pool(name="dram", bufs=1,
                                              space="DRAM"))
        const = ctx.enter_context(tc.tile_pool(name="const", bufs=1))
        wpool = ctx.enter_context(tc.tile_pool(name="wpool", bufs=1))
        xpool = ctx.enter_context(tc.tile_pool(name="xpool", bufs=1))
        state = ctx.enter_context(tc.tile_pool(name="state", bufs=1))
        work = ctx.enter_context(tc.tile_pool(name="work", bufs=1))
        gpool = ctx.enter_context(tc.tile_pool(name="gpool", bufs=3))

        # ---- collectives: reconstruct emb table + direction weights ----
        # (collectives cannot read IO tensors directly; stage via DRAM tiles)
        emb_stage = dram.tile([esh, EMB // 2], mybir.dt.uint8)
        nc.sync.dma_start(emb_stage[:], embsh[:, :])
        w_stage = dram.tile([WSH, G4], FP8)
        nc.sync.dma_start(w_stage[:], wcat[:, :])
        tc.strict_bb_all_engine_barrier()
        # these loads don't depend on the collectives; issue them first so
        # the DMA overlaps the AllGathers (widx ships uint16, widened here)
        widx16 = const.tile([128, NTILE], mybir.dt.uint16)
        nc.sync.dma_start(widx16[:], widx[:, :])
        widx_sb = const.tile([128, NTILE], INT32)
        nc.vector.tensor_copy(widx_sb[:], widx16[:])
        cap8 = xpool.tile([4, TOK], FP8)
        nc.sync.dma_start(cap8[:], caph[:, :])

        emb_full = dram.tile([8 * esh, EMB // 2], mybir.dt.uint8)
        nc.gpsimd.collective_compute(
            "AllGather",
            mybir.AluOpType.bypass,
            replica_groups=[list(range(8))],
            ins=[emb_stage[:].opt()],
            outs=[emb_full[:].opt()],
        )
        wfull = dram.tile([4 * WSH, G4], FP8)
        nc.gpsimd.collective_compute(
            "AllGather",
            mybir.AluOpType.bypass,
            replica_groups=[[0, 1, 2, 3], [4, 5, 6, 7]],
            ins=[w_stage[:].opt()],
            outs=[wfull[:].opt()],
        )
        tc.strict_bb_all_engine_barrier()

        # X^T staging buffer in DRAM (fp8, x8-scaled), filled by the gather
        xt_dram = dram.tile([XTA + XTB, TOK], FP8)

        # ---- constants ----
        id_bf = const.tile([128, 128], BF16)
        id_f32 = const.tile([128, 128], FP32)
        make_identity(nc, id_f32[:])
        nc.vector.tensor_copy(id_bf[:], id_f32[:])
        ones_bf = const.tile([1, 128], BF16)
        nc.gpsimd.memset(ones_bf[:], 1.0)

        # ---- weights -> SBUF: DMA fp8 rows from wfull, dequant to bf16 ----
        def load_w(r0, rows_chunks, scales, nm):
            tiles = []
            for i, (rs, sc) in enumerate(zip(rows_chunks, scales)):
                t8 = wpool.tile([rs, G4], FP8, name=f"wt8_{nm}_{i}")
                nc.sync.dma_start(t8[:], wfull[r0:r0 + rs, :])
                t = wpool.tile([rs, G4], BF16, name=f"wt_{nm}_{i}")
                nc.scalar.activation(t[:], t8[:],
                                     mybir.ActivationFunctionType.Copy,
                                     scale=sc)
                tiles.append(t)
                r0 += rs
            return tiles

        w0a, w0b, w0c, w0d = load_w(
            0, [128, 76, 128, 128], [1 / 64, 1 / 64, 1 / 8, 1 / 8], "w0")
        if has_bias:
            w1a, w1b, w1bias, w1c, w1d = load_w(
                460, [128, 128, 1, 128, 128], [1 / 8] * 5, "w1")
            w2a, w2b, w2bias, w2c, w2d = load_w(
                460 + wr, [128, 128, 1, 128, 128], [1 / 8] * 5, "w2")
        else:
            w1a, w1b, w1c, w1d = load_w(
                460, [128, 128, 128, 128], [1 / 8] * 4, "w1")
            w2a, w2b, w2c, w2d = load_w(
                460 + wr, [128, 128, 128, 128], [1 / 8] * 4, "w2")
            w1bias = w2bias = None

        # ---- cap/ones rows -> xt_dram[200:204, :] ----
        nc.sync.dma_start(xt_dram[200:204, :], cap8[:])

        # ---- embedding gather + transpose into xt_dram[0:200, :] ----
        # (the DGE offset table must be a physical AP, so this loop is
        # python-unrolled; fp8 transpose + DMA-from-PSUM keep it at 5
        # instructions per 128-token tile)
        # int4 dequant affine: value_x8 = (code - 7.5) * delta * 8
        dq_s = float(delta * 8.0)
        dq_b = float(-7.5 * delta * 8.0)

        with tc.tile_pool(name="pprep", bufs=2, space="PSUM") as pprep:
            # pairs of 128-token tiles share one PSUM->SBUF copy + one DMA
            # per feature chunk; each gathered byte holds two int4 codes
            # (feature 2j low nibble, 2j+1 high nibble)
            for i in range((NTILE + 1) // 2):
                js = [2 * i] + ([2 * i + 1] if 2 * i + 1 < NTILE else [])
                n = 128 * len(js)
                # both tiles of the pair gather into one buffer so the 4
                # unpack ops cover the pair at once
                m = (EMB // 2) * len(js)
                g8p = gpool.tile([128, EMB], mybir.dt.uint8,
                                 name="g8", tag="g8")
                for u, j in enumerate(js):
                    nc.gpsimd.indirect_dma_start(
                        out=g8p[:, (EMB // 2) * u:(EMB // 2) * (u + 1)],
                        out_offset=None,
                        in_=emb_full[:, :],
                        in_offset=IndirectOffsetOnAxis(
                            ap=widx_sb[:, j:j + 1], axis=0),
                    )
                lo = gpool.tile([128, EMB], mybir.dt.uint8,
                                name="lo", tag="lo")
                nc.vector.tensor_scalar(lo[:, 0:m], g8p[:, 0:m], 15, None,
                                        op0=mybir.AluOpType.bitwise_and)
                hi = gpool.tile([128, EMB], mybir.dt.uint8,
                                name="hi", tag="hi")
                nc.vector.tensor_scalar(
                    hi[:, 0:m], g8p[:, 0:m], 4, None,
                    op0=mybir.AluOpType.logical_shift_right)
                gb = gpool.tile([128, 2 * EMB], BF16, name="gb", tag="gb")
                gbv = gb[:].rearrange("p (c two) -> p c two", two=2)
                nc.vector.tensor_scalar(gbv[:, 0:m, 0], lo[:, 0:m],
                                        dq_s, dq_b,
                                        op0=mybir.AluOpType.mult,
                                        op1=mybir.AluOpType.add)
                nc.vector.tensor_scalar(gbv[:, 0:m, 1], hi[:, 0:m],
                                        dq_s, dq_b,
                                        op0=mybir.AluOpType.mult,
                                        op1=mybir.AluOpType.add)
                tp1 = pprep.tile([128, 256], BF16, name="tp1", tag="tp1")
                tp2 = pprep.tile([72, 256], BF16, name="tp2", tag="tp2")
                for u in range(len(js)):
                    nc.tensor.transpose(tp1[:, 128 * u:128 * (u + 1)],
                                        gb[:, EMB * u:EMB * u + 128],
                                        id_bf[:])
                    nc.tensor.transpose(tp2[:, 128 * u:128 * (u + 1)],
                                        gb[:, EMB * u + 128:EMB * (u + 1)],
                                        id_bf[:])
                s1 = gpool.tile([128, 256], FP8, name="s1", tag="s1")
                nc.vector.tensor_copy(s1[:, 0:n], tp1[:, 0:n])
                nc.sync.dma_start(
                    xt_dram[0:128, 128 * js[0]:128 * js[0] + n], s1[:, 0:n])
                s2 = gpool.tile([72, 256], FP8, name="s2", tag="s2")
                nc.vector.tensor_copy(s2[:, 0:n], tp2[:, 0:n])
                nc.sync.dma_start(
                    xt_dram[128:200, 128 * js[0]:128 * js[0] + n], s2[:, 0:n])

        tc.strict_bb_all_engine_barrier()

        # ---- recurrent state (fixed addresses) ----
        c_all = state.tile([96, H], FP32)
        nc.gpsimd.memset(c_all[:], 0.0)
        maxht = state.tile([128, 2, BQ], BF16)
        nc.gpsimd.memset(maxht[:], -10.0)
        ht_a = state.tile([128, 2, 96], BF16)   # ht before even ticks
        nc.gpsimd.memset(ht_a[:], 0.0)
        ht_b = state.tile([128, 2, 96], BF16)   # ht before odd ticks

        # peel region X^T: ticks 0..3 use cols 0:128 (fp8 -> bf16, still x8)
        xp_a8 = xpool.tile([XTA, 4 * BQ], FP8)
        nc.sync.dma_start(xp_a8[:], xt_dram[0:XTA, 0:4 * BQ])
        xp_b8 = xpool.tile([XTB, 4 * BQ], FP8)
        nc.sync.dma_start(xp_b8[:], xt_dram[XTA:XTA + XTB, 0:4 * BQ])
        xp_a = xpool.tile([XTA, 4 * BQ], BF16)
        nc.vector.tensor_copy(xp_a[:], xp_a8[:])
        xp_b = xpool.tile([XTB, 4 * BQ], BF16)
        nc.vector.tensor_copy(xp_b[:], xp_b8[:])

        # body X^T slices (refilled by DMA + converted each iteration)
        xc_a8 = xpool.tile([XTA, 2 * BQ], FP8)
        xc_b8 = xpool.tile([XTB, 2 * BQ], FP8)
        xc_a = xpool.tile([XTA, 2 * BQ], BF16)
        xc_b = xpool.tile([XTB, 2 * BQ], BF16)

        # per-parity work tiles (fixed addresses, reused every iteration)
        wt = {}
        for par in (0, 1):
            wt[par] = dict(
                gates=work.tile([96, G4], FP32, name=f"gates{par}"),
                t1=work.tile([96, H], FP32, name=f"t1_{par}"),
                th=work.tile([96, H], FP32, name=f"th_{par}"),
                h_all=work.tile([96, H], BF16, name=f"h_all{par}"),
            )

        def layer_chunks(l, ht, xa, xb):
            if l == 0:
                return [
                    (xa, w0a),
                    (xb, w0b),
                    (ht[:, 0, 0:32], w0c),
                    (ht[:, 1, 0:32], w0d),
                ]
            wa, wb, wbias, wc, wd = (
                (w1a, w1b, w1bias, w1c, w1d) if l == 1 else
                (w2a, w2b, w2bias, w2c, w2d))
            xs = slice(32 * (l - 1), 32 * l)
            hs = slice(32 * l, 32 * (l + 1))
            chunks = [
                (ht[:, 0, xs], wa),
                (ht[:, 1, xs], wb),
                (ht[:, 0, hs], wc),
                (ht[:, 1, hs], wd),
            ]
            if has_bias:
                chunks.insert(2, (ones_bf[0:1, 0:32], wbias))
            return chunks

        def emit_x_parts(z, xa, xb):
            for half in range(2):
                ns = slice(HALF * half, HALF * (half + 1))
                for k, (lhsT, rhs) in enumerate(((xa, w0a), (xb, w0b))):
                    nc.tensor.matmul(z[0:32, ns], lhsT, rhs[:, ns],
                                     start=(k == 0), stop=False,
                                     skip_group_check=True)

        def emit_tick(*, lo, hi, z, xa, xb, ht_in, ht_out, par,
                      x_pre_emitted, zero_tail, do_max):
            if lo == 0:
                rlist = [slice(0, 32 * (hi + 1))]
            else:
                rlist = [slice(32 * l, 32 * (l + 1)) for l in range(lo, hi + 1)]

            lchunks = {}
            for l in range(lo, hi + 1):
                ch = layer_chunks(l, ht_in, xa, xb)
                if l == 0:
                    if x_pre_emitted:
                        ch = ch[2:]
                        starts = [False] * len(ch)
                    else:
                        starts = [k == 0 for k in range(len(ch))]
                else:
                    starts = [k == 0 for k in range(len(ch))]
                lchunks[l] = [(lhsT, rhs, st, k == len(ch) - 1)
                              for k, ((lhsT, rhs), st) in
                              enumerate(zip(ch, starts))]
            maxk = max(len(v) for v in lchunks.values())
            for half in range(2):
                ns = slice(HALF * half, HALF * (half + 1))
                for k in range(maxk):
                    for l in range(lo, hi + 1):
                        chunks = lchunks[l]
                        if k >= len(chunks):
                            continue
                        lhsT, rhs, st, sp = chunks[k]
                        nc.tensor.matmul(
                            z[32 * l:32 * (l + 1), ns], lhsT, rhs[:, ns],
                            start=st, stop=sp, skip_group_check=True)

            w = wt[par]
            gates, t1, th, h_all = w["gates"], w["t1"], w["th"], w["h_all"]
            for r in rlist:
                if has_bias:
                    nc.scalar.activation(gates[r, 0:768], z[r, 0:768],
                                         mybir.ActivationFunctionType.Sigmoid)
                else:
                    nc.scalar.activation(gates[r, SL_F], z[r, SL_F],
                                         mybir.ActivationFunctionType.Sigmoid,
                                         bias=1.0)
                    nc.scalar.activation(gates[r, 256:768], z[r, 256:768],
                                         mybir.ActivationFunctionType.Sigmoid)
                nc.scalar.activation(gates[r, SL_J], z[r, SL_J],
                                     mybir.ActivationFunctionType.Tanh)
                nc.vector.tensor_tensor(c_all[r], gates[r, SL_F], c_all[r],
                                        op=mybir.AluOpType.mult)
                nc.vector.tensor_tensor(t1[r], gates[r, SL_I], gates[r, SL_J],
                                        op=mybir.AluOpType.mult)
                nc.vector.tensor_tensor(c_all[r], c_all[r], t1[r],
                                        op=mybir.AluOpType.add)
                nc.scalar.activation(th[r], c_all[r],
                                     mybir.ActivationFunctionType.Tanh)
                nc.vector.tensor_tensor(h_all[r], gates[r, SL_O], th[r],
                                        op=mybir.AluOpType.mult)
            if zero_tail:
                for rz in range(hi + 1, 3):
                    nc.vector.memset(h_all[32 * rz:32 * (rz + 1), :], 0.0)

            for c in range(2):
                tp = pht.tile([128, 96], BF16, name=f"htpp{par}{c}",
                              tag=f"htpp{par}{c}")
                nc.tensor.transpose(tp[:], h_all[:, 128 * c:128 * (c + 1)],
                                    id_bf[0:96, 0:96])
                nc.vector.tensor_copy(ht_out[:, c, :], tp[:])

            if do_max:
                nc.vector.tensor_tensor(maxht[:], maxht[:],
                                        ht_out[:, :, 64:96],
                                        op=mybir.AluOpType.max)

        with tc.tile_pool(name="pz", bufs=1, space="PSUM") as pz, \
             tc.tile_pool(name="pht", bufs=1, space="PSUM") as pht:
            zE = pz.tile([96, G4], FP32, name="zE")
            zO = pz.tile([96, G4], FP32, name="zO")

            # ---- peel ticks 0..3 ----
            for tau in range(4):
                par = tau % 2
                z = (zE, zO)[par]
                ht_in, ht_out = ((ht_a, ht_b), (ht_b, ht_a))[par]
                xa = xp_a[:, BQ * tau:BQ * (tau + 1)]
                xb = xp_b[:, BQ * tau:BQ * (tau + 1)]
                emit_tick(lo=0, hi=min(2, tau), z=z, xa=xa, xb=xb,
                          ht_in=ht_in, ht_out=ht_out, par=par,
                          x_pre_emitted=False, zero_tail=(tau < 2),
                          do_max=(tau >= 2))

            # ---- hardware loop: ticks 4..T-1, two per iteration ----
            with tc.For_i(4 * BQ, TOK, 2 * BQ) as iv:
                nc.sync.dma_start(xc_a8[:], xt_dram[0:XTA, ds(iv, 2 * BQ)])
                nc.sync.dma_start(xc_b8[:],
                                  xt_dram[XTA:XTA + XTB, ds(iv, 2 * BQ)])
                nc.vector.tensor_copy(xc_a[:], xc_a8[:])
                nc.vector.tensor_copy(xc_b[:], xc_b8[:])
                emit_x_parts(zE, xc_a[:, 0:BQ], xc_b[:, 0:BQ])
                emit_x_parts(zO, xc_a[:, BQ:2 * BQ], xc_b[:, BQ:2 * BQ])
                emit_tick(lo=0, hi=2, z=zE,
                          xa=xc_a[:, 0:BQ], xb=xc_b[:, 0:BQ],
                          ht_in=ht_a, ht_out=ht_b, par=0,
                          x_pre_emitted=True, zero_tail=False, do_max=True)
                emit_tick(lo=0, hi=2, z=zO,
                          xa=xc_a[:, BQ:2 * BQ], xb=xc_b[:, BQ:2 * BQ],
                          ht_in=ht_b, ht_out=ht_a, par=1,
                          x_pre_emitted=True, zero_tail=False, do_max=True)

            # ---- cooldown ticks T, T+1 ----
            for tau in (T, T + 1):
                par = tau % 2
                z = (zE, zO)[par]
                ht_in, ht_out = ((ht_a, ht_b), (ht_b, ht_a))[par]
                emit_tick(lo=tau - (T - 1), hi=2, z=z, xa=None, xb=None,
                          ht_in=ht_in, ht_out=ht_out, par=par,
                          x_pre_emitted=False, zero_tail=False, do_max=True)

        nc.sync.dma_start(
            out[:, :].rearrange("p (c rr) -> p c rr", c=2), maxht[:, :, :])

    nc.finalize()
    return nc


_NC_CACHE = {}
TRACE = False
LAST_RESULTS = None
LAST_RUN_WALL_S = None


def _get_program(T, esh, delta, has_bias=True):
    key = (T, esh, float(delta), has_bias)
    if key not in _NC_CACHE:
        _NC_CACHE[key] = _build_program(T, esh, delta, has_bias=has_bias)
    return _NC_CACHE[key]


def _gate_perm():
    # TF order [i, j, f, o] (256 each) -> [f, i, o, j]
    i = np.arange(0, 256)
    j = np.arange(256, 512)
    f = np.arange(512, 768)
    o = np.arange(768, 1024)
    return np.concatenate([f, i, o, j])


def _prep_lstm_w(W, b, perm, layer0, has_bias):
    Wp = np.asarray(W, np.float32)[:, perm]
    bp = np.asarray(b, np.float32)[perm].copy()
    if has_bias:
        # forget_bias folded into the sigmoid argument; in the no-bias path
        # the device's ScalarE bias=1.0 covers it for every layer
        bp[0:256] += 1.0
    if layer0:
        stacked = np.concatenate(
            [Wp[0:203], bp[None, :], Wp[203:459]], axis=0)
        assert stacked.shape[0] == 460
    elif has_bias:
        stacked = np.concatenate([Wp[0:256], bp[None, :], Wp[256:512]], axis=0)
        assert stacked.shape[0] == 513
    else:
        stacked = Wp
        assert stacked.shape[0] == 512
    return stacked


def _elu(x):
    return np.where(x > 0, x, np.expm1(np.minimum(x, 0.0)))


def kernel(**inputs):
    import ml_dtypes

    words = np.asarray(inputs["words"])
    capitals = np.asarray(inputs["capitals"])
    B, T = words.shape
    assert B == B_FULL

    perm = _gate_perm()
    cap_table = np.asarray(inputs["cap_table"], np.float32)
    hb = any(np.any(np.asarray(inputs[k], np.float32) != 0.0)
             for k in ("bf1", "bf2", "bb1", "bb2"))

    # vocab compaction: ship only embedding rows some core actually reads
    uniq = np.unique(words)                     # sorted unique token ids
    words_c = np.searchsorted(uniq, words).astype(np.int32)
    esh = -(-len(uniq) // 8)                    # shard rows per core
    emb_f32 = np.asarray(inputs["embed_words"], np.float32)
    delta = 0.3352 * float(emb_f32.std())       # 16-level uniform quantizer
    nc = _get_program(T, esh, delta, hb)

    F8 = ml_dtypes.float8_e4m3
    # per-direction weight stacks, shipped as 1/4 shards within each group
    w_by_dir = []
    for d, (k0, k1, k2) in enumerate(
            [("Wf0", "Wf1", "Wf2"), ("Wb0", "Wb1", "Wb2")]):
        b0, b1, b2 = ("bf0", "bf1", "bf2") if d == 0 else ("bb0", "bb1", "bb2")
        wall = np.concatenate([
            _prep_lstm_w(inputs[k0], inputs[b0], perm, True, hb),
            _prep_lstm_w(inputs[k1], inputs[b1], perm, False, hb),
            _prep_lstm_w(inputs[k2], inputs[b2], perm, False, hb),
        ], axis=0)
        pad = 4 * WSH - wall.shape[0]
        wall = np.concatenate([wall, np.zeros((pad, G4), np.float32)], axis=0)
        w_by_dir.append((wall * 8).astype(F8))

    # int4 quantize + pack two codes per byte (feat 2j low, 2j+1 high)
    emb_c = emb_f32[uniq]
    pad = 8 * esh - emb_c.shape[0]
    if pad:
        emb_c = np.concatenate(
            [emb_c, np.zeros((pad, EMB), np.float32)], axis=0)
    code = np.clip(np.round(emb_c / delta + 7.5), 0, 15).astype(np.uint8)
    emb_q8 = (code[:, 0::2] | (code[:, 1::2] << 4)).astype(np.uint8)
    capt_q8 = (cap_table * 8).astype(F8)
    TOK = BQ * T

    in_maps = []
    for p in range(8):
        d, q = p // 4, p % 4
        wl = words_c[BQ * q:BQ * (q + 1)]
        cl = capitals[BQ * q:BQ * (q + 1)]
        if d == 1:
            wl = wl[:, ::-1]
            cl = cl[:, ::-1]
        wflat = np.ascontiguousarray(wl.T).reshape(-1)   # r = t*32 + b
        ntile = wflat.shape[0] // 128
        assert len(uniq) < 65536
        widx_np = np.ascontiguousarray(
            wflat.reshape(ntile, 128).T).astype(np.uint16)
        cflat = cl.T.reshape(-1)
        caph_np = np.empty((4, TOK), F8)
        caph_np[0:3] = capt_q8[cflat].T
        caph_np[3] = np.float32(8.0)

        in_maps.append({
            "embsh": np.ascontiguousarray(emb_q8[esh * p:esh * (p + 1)]),
            "widx": widx_np,
            "caph": caph_np,
            "wcat": np.ascontiguousarray(
                w_by_dir[d][WSH * q:WSH * (q + 1)]),
        })

    global LAST_RESULTS, LAST_RUN_WALL_S
    import time as _time
    kwargs = {}
    if TRACE:
        kwargs = dict(trace=True, trace_cores=list(range(8)))
    _t0 = _time.time()
    try:
        res = run_bass_kernel_spmd(nc, in_maps, core_ids=list(range(8)), **kwargs)
    except Exception:
        if not kwargs:
            raise
        res = run_bass_kernel_spmd(nc, in_maps, core_ids=list(range(8)))
    LAST_RUN_WALL_S = _time.time() - _t0
    LAST_RESULTS = res

    rnn_out = np.empty((B_FULL, 2 * H), np.float32)
    for p in range(8):
        d, q = p // 4, p % 4
        mh = np.asarray(res.results[p]["out"]).astype(np.float32)
        mh = mh.reshape(128, 2, BQ)
        for c in range(2):
            rnn_out[BQ * q:BQ * (q + 1),
                    256 * d + 128 * c:256 * d + 128 * (c + 1)] = mh[:, c, :].T
    d1_W = np.asarray(inputs["d1_W"], np.float32)
    d1_b = np.asarray(inputs["d1_b"], np.float32)
    d2_W = np.asarray(inputs["d2_W"], np.float32)
    d2_b = np.asarray(inputs["d2_b"], np.float32)
    h1 = _elu(rnn_out @ d1_W + d1_b)
    out = 1.0 / (1.0 + np.exp(-(h1 @ d2_W + d2_b)))
    return out.astype(np.float32)
